# revision 5
# baseline (speedup 1.0000x reference)
"""MI-LSTM (attention LSTM) + LSTM + linear head for Trainium2, 8-core batch-parallel.

Model (per timestep, per batch row b):
  gm = y@W_main + h@U_main + b_main -> i,f,o,cm gates
  ga[k] = x_k@W_aux[k] + h@U_aux[k] + b_aux[k] -> i_k (sigmoid), c_k (tanh)
  candidates l = [i*cm, i_k*c_k] (9, H)
  u_k = tanh(l_k . (W_att @ c) + b_att); a = softmax(u); L = sum a_k l_k
  c' = f*c + L; h' = o*tanh(c')
Then a standard LSTM over the h-sequence, then relu + linear to scalar.

Mapping: batch sharded 8 ways (256 rows/core = 2 partition tiles of 128).
Phase-1 attention math is batch-major (batch on partitions). The x/y inputs
are pre-transposed to feature-major on the HOST (with a built-in ones row
for the biases) so the in-loop transposes are only for h/c. The gate matmul
accumulates an x-part and an h-part directly in PSUM. Phase 2 (standard
LSTM) runs feature-major (gates on partitions) interleaved one step behind
phase 1 so its work fills phase-1 dependency stalls; its h2 state feeds the
next matmul directly with no transposes. exp() for softmax is computed as
(1+t)/(1-t) with t=tanh(u/2) so only one ACT table set is ever loaded.
"""

import os
import numpy as np
import ml_dtypes

import concourse.bacc as bacc
import concourse.bass as bass
import concourse.mybir as mybir
from concourse.tile import TileContext
from concourse.bass_utils import run_bass_kernel_spmd

F32 = mybir.dt.float32
BF16 = mybir.dt.bfloat16
ALU = mybir.AluOpType
ACTF = mybir.ActivationFunctionType
AX = mybir.AxisListType

S, B, F, H, K = 256, 2048, 5, 64, 8
NC = 8
BL = B // NC          # 256 batch rows per core
NT = BL // 128        # 2 partition tiles
NCAND = K + 1         # 9 candidates
XR_ROWS = 46          # 5 y + 40 x + 1 ones (bias row)
CH = 8                # steps per x-slab DMA chunk

LAST_RESULTS = {}


def _build(n_steps: int, b_att: float):
    nc = bacc.Bacc(None, target_bir_lowering=False)

    xin = nc.dram_tensor("xin", [n_steps, XR_ROWS, BL], BF16, kind="ExternalInput")
    w46 = nc.dram_tensor("w46", [XR_ROWS, 1280], BF16, kind="ExternalInput")
    wh = nc.dram_tensor("wh", [H, 1280], BF16, kind="ExternalInput")
    watt = nc.dram_tensor("watt", [H, H], BF16, kind="ExternalInput")
    wca2 = nc.dram_tensor("wca2", [H + 1, 4 * H], BF16, kind="ExternalInput")
    wcb = nc.dram_tensor("wcb", [H, 4 * H], BF16, kind="ExternalInput")
    linw = nc.dram_tensor("linw", [H, 1], BF16, kind="ExternalInput")
    idf32 = nc.dram_tensor("idf32", [128, 128], F32, kind="ExternalInput")
    onesrow = nc.dram_tensor("onesrow", [1, (n_steps + 1) * BL], BF16,
                             kind="ExternalInput")
    out = nc.dram_tensor("out", [n_steps, BL, 1], F32, kind="ExternalOutput")

    n_half = (n_steps + 127) // 128  # OACC column blocks

    with TileContext(nc) as tc:
        with (
            tc.tile_pool(name="state", bufs=1) as st,
            tc.tile_pool(name="wts", bufs=1) as wp,
            tc.tile_pool(name="work", bufs=2) as wk,
            tc.tile_pool(name="xr", bufs=2) as xrp,
            tc.tile_pool(name="gpsum", bufs=1, space="PSUM") as gp,
            tc.tile_pool(name="mpsum", bufs=1, space="PSUM") as mp,
            tc.tile_pool(name="p2psum", bufs=1, space="PSUM") as p2p,
            tc.tile_pool(name="hdpsum", bufs=1, space="PSUM") as hdp,
        ):
            # ---- persistent weights in SBUF ----
            W46 = wp.tile([XR_ROWS, 1280], BF16, tag="w46")
            WH = wp.tile([H, 1280], BF16, tag="wh")
            WA = wp.tile([H, H], BF16, tag="watt")
            WCA2 = wp.tile([H + 1, 4 * H], BF16, tag="wca2")
            WCB = wp.tile([H, 4 * H], BF16, tag="wcb")
            LW = wp.tile([H, 1], BF16, tag="linw")
            IDF = wp.tile([128, 128], F32, tag="idf32")
            for t_, d_ in ((W46, w46), (WH, wh), (WA, watt), (WCA2, wca2),
                           (WCB, wcb), (LW, linw), (IDF, idf32)):
                nc.sync.dma_start(t_[:], d_[:])

            # ---- persistent state ----
            HC1 = st.tile([128, 2 * 128], F32, tag="hc1")    # [h|c] per tau
            HST = st.tile([H + 1, (n_steps + 1) * BL], BF16, tag="hst")
            H2T = st.tile([H, BL], BF16, tag="h2t")          # phase-2 h^T
            C2 = st.tile([H, BL], F32, tag="c2")             # phase-2 c
            OACC = st.tile([128, n_half * BL], F32, tag="oacc")

            nc.vector.memset(HC1[:], 0.0)
            nc.vector.memset(H2T[:], 0.0)
            nc.vector.memset(C2[:], 0.0)
            nc.vector.memset(HST[0:H, 0:BL], 0.0)            # slot 0: h(-1)=0
            nc.sync.dma_start(HST[H:H + 1, :], onesrow[:])   # bias row

            xch = xin.rearrange("(c n) r b -> c r n b", n=CH)
            n_chunks = (n_steps + CH - 1) // CH
            xr_tiles = {}
            for c in range(min(2, n_chunks)):
                xr_tiles[c] = xrp.tile([XR_ROWS, CH * BL], BF16, tag="xr")
                nc.sync.dma_start(xr_tiles[c][:].rearrange("r (n b) -> r n b", n=CH), xch[c])

            def phase2_step(u):
                """Standard LSTM step u (needs HST slot u+1 = h1(u))."""
                hs = HST[:, (u + 1) * BL:(u + 2) * BL]
                g2 = p2p.tile([128, 2 * BL], F32, tag="g2")
                # P0 = [i; f] gates, P1 = [o; g] gates (feature-major)
                for p_ in range(2):
                    o0 = p_ * BL
                    nc.tensor.matmul(g2[:, o0:o0 + BL], WCA2[:, p_ * 128:(p_ + 1) * 128],
                                     hs, start=True, stop=False)
                    nc.tensor.matmul(g2[:, o0:o0 + BL], WCB[:, p_ * 128:(p_ + 1) * 128],
                                     H2T[:], start=False, stop=True)
                s2 = wk.tile([128, BL], BF16, tag="s2")      # [i; f]
                o2 = wk.tile([H, BL], BF16, tag="o2")
                g2t = wk.tile([H, BL], BF16, tag="g2t")
                nc.scalar.activation(s2[:], g2[:, 0:BL], ACTF.Sigmoid)
                nc.scalar.activation(o2[:], g2[0:H, BL:2 * BL], ACTF.Sigmoid)
                nc.scalar.activation(g2t[:], g2[H:128, BL:2 * BL], ACTF.Tanh)
                ig = wk.tile([H, BL], BF16, tag="ig")
                nc.vector.tensor_mul(ig[:], s2[0:H, :], g2t[:])
                fc2 = wk.tile([H, BL], F32, tag="fc2")
                nc.vector.tensor_mul(fc2[:], s2[H:128, :], C2[:])
                nc.vector.tensor_tensor(C2[:], ig[:], fc2[:], ALU.add)
                tc2 = wk.tile([H, BL], BF16, tag="tc2")
                nc.scalar.activation(tc2[:], C2[:], ACTF.Tanh)
                nc.vector.tensor_mul(H2T[:], o2[:], tc2[:])
                rh = wk.tile([H, BL], BF16, tag="rh")
                nc.vector.tensor_scalar_max(rh[:], H2T[:], 0.0)
                hd = hdp.tile([1, BL], F32, tag="hd")
                nc.tensor.matmul(hd[:], LW[:], rh[:], start=True, stop=True)
                r_ = u % 128
                cb = (u // 128) * BL
                nc.scalar.activation(OACC[r_:r_ + 1, cb:cb + BL], hd[:], ACTF.Copy)

            # ================= main loop =================
            for t in range(n_steps):
                # -- h/c transposes: misc cols 0:256; v matmul: cols 256:384
                misc = mp.tile([128, 512], F32, tag="misc")
                for tau in range(NT):
                    nc.tensor.transpose(
                        misc[0:128, tau * 128:(tau + 1) * 128],
                        HC1[:, tau * 128:(tau + 1) * 128], IDF[:])
                # h^T -> HST slot t (= h1(t-1)); c^T -> CT
                nc.scalar.activation(HST[0:H, t * BL:(t + 1) * BL],
                                     misc[0:H, 0:256], ACTF.Copy)
                CT = wk.tile([H, BL], BF16, tag="ct")
                nc.scalar.activation(CT[:], misc[H:128, 0:256], ACTF.Copy)

                # v = (W_att @ c)^T, batch-major [128, (tau, h)]
                for tau in range(NT):
                    nc.tensor.matmul(misc[:, 256 + tau * H:256 + (tau + 1) * H],
                                     CT[:, tau * 128:(tau + 1) * 128], WA[:],
                                     start=True, stop=True)
                vS = wk.tile([128, NT * H], BF16, tag="vs")
                nc.scalar.activation(vS[:], misc[:, 256:384], ACTF.Copy)

                # -- gate matmuls: x-part + h-part accumulate in PSUM
                ch, sl = t // CH, t % CH
                XR = xr_tiles[ch]
                if sl == 0 and ch + 1 < n_chunks and (ch + 1) not in xr_tiles:
                    xr_tiles[ch + 1] = xrp.tile([XR_ROWS, CH * BL], BF16, tag="xr")
                    nc.sync.dma_start(xr_tiles[ch + 1][:].rearrange("r (n b) -> r n b", n=CH), xch[ch + 1])
                xr_tiles.pop(ch - 1, None)
                hsl = HST[0:H, t * BL:(t + 1) * BL]
                gps = gp.tile([128, 2560], F32, tag="gates")
                for tau in range(NT):
                    xl = XR[:, sl * BL + tau * 128:sl * BL + (tau + 1) * 128]
                    hl = hsl[:, tau * 128:(tau + 1) * 128]
                    for (o0, w0, w1) in ((tau * 512, 0, 512),
                                         (1024 + tau * 512, 512, 1024),
                                         (2048 + tau * 256, 1024, 1280)):
                        nc.tensor.matmul(gps[:, o0:o0 + (w1 - w0)], xl,
                                         W46[:, w0:w1], start=True, stop=False)
                        nc.tensor.matmul(gps[:, o0:o0 + (w1 - w0)], hl,
                                         WH[:, w0:w1], start=False, stop=True)

                # -- activations (PSUM -> SBUF bf16), layout [128,(t,cand,h)]
                sig = wk.tile([128, NT * 576], BF16, tag="sig")
                tan = wk.tile([128, NT * 576], BF16, tag="tau")
                fo = wk.tile([128, NT * 128], BF16, tag="fo")
                sigA = gps[:, 0:1024].rearrange("p (t c) -> p t c", t=2)
                tanA = gps[:, 1024:2048].rearrange("p (t c) -> p t c", t=2)
                mn = gps[:, 2048:2560].rearrange("p (t c) -> p t c", t=2)
                sigv = sig[:].rearrange("p (t c) -> p t c", t=2)
                tanv = tan[:].rearrange("p (t c) -> p t c", t=2)
                nc.scalar.activation(sigv[:, :, 64:576], sigA, ACTF.Sigmoid)
                nc.scalar.activation(tanv[:, :, 64:576], tanA, ACTF.Tanh)
                nc.scalar.activation(sigv[:, :, 0:64], mn[:, :, 0:64], ACTF.Sigmoid)
                nc.scalar.activation(
                    fo[:].rearrange("p (t c) -> p t c", t=2),
                    mn[:, :, 64:192], ACTF.Sigmoid)
                nc.scalar.activation(tanv[:, :, 0:64], mn[:, :, 192:256], ACTF.Tanh)

                # -- candidates l, attention scores u
                l_t = wk.tile([128, NT * 576], BF16, tag="l")
                nc.vector.tensor_mul(l_t[:], sig[:], tan[:])
                z_t = wk.tile([128, NT * 576], BF16, tag="z")
                vb = (vS[:].rearrange("p (t h) -> p t h", t=2)
                      .unsqueeze(2).broadcast_to((128, 2, NCAND, H)))
                nc.vector.tensor_tensor(
                    z_t[:].rearrange("p (t k h) -> p t k h", k=NCAND, h=H),
                    l_t[:].rearrange("p (t k h) -> p t k h", k=NCAND, h=H),
                    vb, ALU.mult)
                # u-tree: reduce h by strided halves (bf16 2x mode)
                zv = z_t[:].rearrange("p (t k h) -> p t k h", k=NCAND, h=H)
                zt1 = wk.tile([128, NT * NCAND * 32], BF16, tag="zt1")
                nc.vector.tensor_tensor(
                    zt1[:].rearrange("p (t k h) -> p t k h", k=NCAND, h=32),
                    zv[:, :, :, 0:32], zv[:, :, :, 32:64], ALU.add)
                z1v = zt1[:].rearrange("p (t k h) -> p t k h", k=NCAND, h=32)
                zt2 = wk.tile([128, NT * NCAND * 16], BF16, tag="zt2")
                nc.vector.tensor_tensor(
                    zt2[:].rearrange("p (t k h) -> p t k h", k=NCAND, h=16),
                    z1v[:, :, :, 0:16], z1v[:, :, :, 16:32], ALU.add)
                z2v = zt2[:].rearrange("p (t k h) -> p t k h", k=NCAND, h=16)
                zt3 = wk.tile([128, NT * NCAND * 8], BF16, tag="zt3")
                nc.vector.tensor_tensor(
                    zt3[:].rearrange("p (t k h) -> p t k h", k=NCAND, h=8),
                    z2v[:, :, :, 0:8], z2v[:, :, :, 8:16], ALU.add)
                z3v = zt3[:].rearrange("p (t k h) -> p t k h", k=NCAND, h=8)
                zt4 = wk.tile([128, NT * NCAND * 4], BF16, tag="zt4")
                nc.vector.tensor_tensor(
                    zt4[:].rearrange("p (t k h) -> p t k h", k=NCAND, h=4),
                    z3v[:, :, :, 0:4], z3v[:, :, :, 4:8], ALU.add)
                z4v = zt4[:].rearrange("p (t k h) -> p t k h", k=NCAND, h=4)
                zt5 = wk.tile([128, NT * NCAND * 2], BF16, tag="zt5")
                nc.vector.tensor_tensor(
                    zt5[:].rearrange("p (t k h) -> p t k h", k=NCAND, h=2),
                    z4v[:, :, :, 0:2], z4v[:, :, :, 2:4], ALU.add)
                z5v = zt5[:].rearrange("p (t k h) -> p t k h", k=NCAND, h=2)
                u_t = wk.tile([128, NT * NCAND], F32, tag="u")
                nc.vector.tensor_tensor(
                    u_t[:].rearrange("p (t k) -> p t k 1", k=NCAND),
                    z5v[:, :, :, 0:1], z5v[:, :, :, 1:2], ALU.add)

                # softmax via exp(u) = (1+t2)/(1-t2), t2 = tanh(u/2)
                ut2 = wk.tile([128, NT * NCAND], F32, tag="ut2")
                nc.scalar.activation(ut2[:], u_t[:], ACTF.Tanh, bias=b_att, scale=1.0)
                t2 = wk.tile([128, NT * NCAND], F32, tag="t2")
                nc.scalar.activation(t2[:], ut2[:], ACTF.Tanh, scale=0.5)
                q_t = wk.tile([128, NT * NCAND], F32, tag="q")
                nc.vector.tensor_scalar(q_t[:], t2[:], -1.0, 1.0, ALU.mult, ALU.add)
                rq = wk.tile([128, NT * NCAND], F32, tag="rq")
                nc.vector.reciprocal_approx_fast(rq[:], q_t[:])
                r_t = wk.tile([128, NT * NCAND], BF16, tag="r")
                nc.vector.scalar_tensor_tensor(
                    r_t[:], t2[:], 1.0, rq[:], ALU.add, ALU.mult)
                s_t = wk.tile([128, NT], F32, tag="s")
                nc.vector.tensor_reduce(
                    s_t[:],
                    r_t[:].rearrange("p (t k) -> p t k", t=2), AX.X, ALU.add)
                rs = wk.tile([128, NT], F32, tag="rs")
                nc.vector.reciprocal_approx_fast(rs[:], s_t[:])
                # rp: r duplicated pairwise so the aw-mul broadcast hits 2x mode
                rp = wk.tile([128, NT * NCAND * 2], BF16, tag="rp")
                rpv = rp[:].rearrange("p (c two) -> p c two", two=2)
                rv1 = r_t[:].rearrange("p c -> p c 1")
                nc.vector.tensor_copy(rpv[:, :, 0:1], rv1)
                nc.vector.tensor_copy(rpv[:, :, 1:2], rv1)

                # aw = l * exp(u) broadcast over h
                aw = wk.tile([128, NT * 576], BF16, tag="aw")
                lp4 = l_t[:].rearrange("p (c h2 two) -> p c h2 two", h2=32, two=2)
                rb4 = (rp[:].rearrange("p (c two) -> p c two", two=2)
                       .unsqueeze(2).broadcast_to((128, NT * NCAND, 32, 2)))
                nc.vector.tensor_tensor(
                    aw[:].rearrange("p (c h2 two) -> p c h2 two", h2=32, two=2),
                    lp4, rb4, ALU.mult)
                # Lp-tree: reduce k (9 = 8+1) by strided halves
                awv = aw[:].rearrange("p (t k h) -> p t k h", k=NCAND, h=H)
                at1 = wk.tile([128, NT * 4 * H], BF16, tag="at1")
                nc.vector.tensor_tensor(
                    at1[:].rearrange("p (t k h) -> p t k h", k=4, h=H),
                    awv[:, :, 0:4, :], awv[:, :, 4:8, :], ALU.add)
                a1v = at1[:].rearrange("p (t k h) -> p t k h", k=4, h=H)
                at2 = wk.tile([128, NT * 2 * H], F32, tag="at2")
                nc.vector.tensor_tensor(
                    at2[:].rearrange("p (t k h) -> p t k h", k=2, h=H),
                    a1v[:, :, 0:2, :], a1v[:, :, 2:4, :], ALU.add)
                a2v = at2[:].rearrange("p (t k h) -> p t k h", k=2, h=H)
                at3 = wk.tile([128, NT * H], F32, tag="at3")
                nc.vector.tensor_tensor(
                    at3[:].rearrange("p (t h) -> p t 1 h", h=H),
                    a2v[:, :, 0:1, :], a2v[:, :, 1:2, :], ALU.add)
                Lp = wk.tile([128, NT * H], F32, tag="Lp")
                nc.vector.tensor_tensor(
                    Lp[:].rearrange("p (t h) -> p t 1 h", h=H),
                    at3[:].rearrange("p (t h) -> p t 1 h", h=H),
                    awv[:, :, 8:9, :], ALU.add)

                # state update: c' = f*c + rs*Lp; h' = o*tanh(c')
                hc1v = HC1[:].rearrange("p (t x) -> p t x", t=2)
                fov = fo[:].rearrange("p (t x) -> p t x", t=2)
                fc = wk.tile([128, NT * H], F32, tag="fc")
                nc.vector.tensor_tensor(
                    fc[:].rearrange("p (t h) -> p t h", t=2),
                    fov[:, :, 0:64], hc1v[:, :, 64:128], ALU.mult)
                for tau in range(NT):
                    nc.vector.scalar_tensor_tensor(
                        HC1[:, tau * 128 + 64:tau * 128 + 128],
                        Lp[:, tau * H:(tau + 1) * H], rs[:, tau:tau + 1],
                        fc[:, tau * H:(tau + 1) * H], ALU.mult, ALU.add)
                tc1 = wk.tile([128, NT * H], BF16, tag="tc1")
                nc.scalar.activation(
                    tc1[:].rearrange("p (t h) -> p t h", t=2),
                    hc1v[:, :, 64:128], ACTF.Tanh)
                nc.vector.tensor_tensor(
                    hc1v[:, :, 0:64],
                    fov[:, :, 64:128],
                    tc1[:].rearrange("p (t h) -> p t h", t=2), ALU.mult)

                # -- phase 2, one step behind
                if t >= 1:
                    phase2_step(t - 1)

            # epilogue: h1(n-1) -> HST slot n, then last phase-2 step
            misc = mp.tile([128, 512], F32, tag="misc")
            for tau in range(NT):
                nc.tensor.transpose(
                    misc[0:128, tau * 128:(tau + 1) * 128],
                    HC1[:, tau * 128:(tau + 1) * 128], IDF[:])
            nc.scalar.activation(HST[0:H, n_steps * BL:(n_steps + 1) * BL],
                                 misc[0:H, 0:256], ACTF.Copy)
            phase2_step(n_steps - 1)

            ov = out.rearrange("(hf p) b one -> p (hf b one)", p=min(128, n_steps))
            nc.sync.dma_start(ov[:], OACC[0:min(128, n_steps), 0:n_half * BL])

    nc.finalize()
    return nc


def _prep_weights(inp):
    f32 = np.float32
    W_main, U_main, b_main = (np.asarray(inp["W_main"], f32),
                              np.asarray(inp["U_main"], f32),
                              np.asarray(inp["b_main"], f32))
    W_aux, U_aux, b_aux = (np.asarray(inp["W_aux"], f32),
                           np.asarray(inp["U_aux"], f32),
                           np.asarray(inp["b_aux"], f32))
    w46 = np.zeros((XR_ROWS, 1280), f32)
    wh = np.zeros((H, 1280), f32)
    for k in range(K):
        c = 64 * k
        w46[5 + 5 * k:10 + 5 * k, c:c + 64] = W_aux[k, :, 0:64]
        w46[45, c:c + 64] = b_aux[k, 0:64]
        wh[:, c:c + 64] = U_aux[k, :, 0:64]
        w46[5 + 5 * k:10 + 5 * k, 512 + c:512 + c + 64] = W_aux[k, :, 64:128]
        w46[45, 512 + c:512 + c + 64] = b_aux[k, 64:128]
        wh[:, 512 + c:512 + c + 64] = U_aux[k, :, 64:128]
    w46[0:5, 1024:1280] = W_main
    w46[45, 1024:1280] = b_main
    wh[:, 1024:1280] = U_main

    watt = np.asarray(inp["W_att"], f32).T.copy()          # (64,64): rhs for v
    # phase 2: cols reordered [i f o | g] (torch gate order i,f,g,o)
    perm = np.concatenate([np.arange(0, 128), np.arange(192, 256),
                           np.arange(128, 192)])
    wca2 = np.zeros((H + 1, 4 * H), f32)
    wca2[0:H] = np.asarray(inp["W_ih"], f32).T[:, perm]
    wca2[H] = (np.asarray(inp["b_ih"], f32) + np.asarray(inp["b_hh"], f32))[perm]
    wcb = np.asarray(inp["W_hh"], f32).T[:, perm].copy()
    linw = np.asarray(inp["lin_W"], f32).reshape(H, 1)

    bf = ml_dtypes.bfloat16
    return dict(
        w46=w46.astype(bf), wh=wh.astype(bf), watt=watt.astype(bf),
        wca2=wca2.astype(bf), wcb=wcb.astype(bf), linw=linw.astype(bf),
        idf32=np.eye(128, dtype=f32),
    )


def kernel(**inputs) -> np.ndarray:
    n_steps = int(os.environ.get("KERNEL_STEPS", S))
    names = ["Y"] + ["x%d" % i for i in range(1, 9)]
    bf = ml_dtypes.bfloat16
    # host-side feature-major slabs: [n_steps, 46, B] with ones row 45
    big = np.empty((n_steps, XR_ROWS, B), np.float32)
    for i, n in enumerate(names):
        a = np.asarray(inputs[n], np.float32)[:n_steps]       # (s, B, F)
        big[:, 5 * i:5 * i + 5, :] = a.transpose(0, 2, 1)
    big[:, 45, :] = 1.0
    big = big.astype(bf)
    wmaps = _prep_weights(inputs)
    b_att = float(np.asarray(inputs["b_att"]).reshape(-1)[0])
    lin_b = float(np.asarray(inputs["lin_b"]).reshape(-1)[0])

    nc = _build(n_steps, b_att)
    ones = np.ones((1, (n_steps + 1) * BL), bf)
    in_maps = []
    for c in range(NC):
        m = dict(wmaps)
        m["xin"] = np.ascontiguousarray(big[:, :, c * BL:(c + 1) * BL])
        m["onesrow"] = ones
        in_maps.append(m)

    trace = bool(int(os.environ.get("KERNEL_TRACE", "0")))
    res = run_bass_kernel_spmd(nc, in_maps, core_ids=list(range(NC)),
                               trace=trace)
    LAST_RESULTS["exec_time_ns"] = res.exec_time_ns
    LAST_RESULTS["trace"] = res.instructions_and_trace

    outs = [r["out"] for r in res.results]  # each (n_steps, BL, 1)
    full = np.concatenate(outs, axis=1) + lin_b
    return full.astype(np.float32)
